# revision 1
# baseline (speedup 1.0000x reference)
"""NT-Xent loss kernel for 8 Trainium2 NeuronCores (Bass/Tile).

Strategy (data-parallel rows, SPMD):
  - Host: concat z_i,z_j -> reps [8192, 512], cast bf16. Core c receives
    np.roll(reps, -c*1024, axis=0) so every core runs the same static
    program on "its" first 1024 rows: self-similarity for local row li
    sits at column li, the positive partner at column li+4096.
  - On-chip per core: row squared-norms via fused DVE multiply+accumulate
    (scalar_tensor_tensor), inv-norm via DVE-only Newton rsqrt (constant
    seed 1/sqrt(512); avoids ScalarE activation-table reloads entirely,
    which cost ~1.3us per Sqrt/Ln<->Exp switch), normalize on DVE,
    transpose into
    rblk-major repsT [p, rblk, sub, k, c]: groups 0-1 via PE transposes
    (fast pipeline start), groups 2-7 via one batched XBAR DMA-transpose
    each (runs on the otherwise-idle Sync engine). Similarity block
    computed as [128, 1024] PSUM tiles (bf16 matmul, f32 accum, 3-dim
    moving AP). Self column masked with a -1e30 eye tile; exp(4*sim-4)
    on ScalarE with fused row-sum accumulation; row-max via running
    elementwise tensor_max (2x bf16) + one final reduce per m-tile.
  - Host: combine per-core stats (positives, hardest negatives, exp sums)
    in float64 into the scalar loss (the two "all-reduced" loss terms).
"""

import numpy as np
import ml_dtypes

import concourse.bacc as bacc
import concourse.bass as bass
import concourse.tile as tile
import concourse.mybir as mybir
from concourse.bass_utils import run_bass_kernel_spmd

B = 4096
D = 512
N = 2 * B            # 8192 rows total
NCORES = 8
NLOC = N // NCORES   # 1024 rows per core
RT = N // 128        # 64 row tiles
MT = NLOC // 128     # 8 local row tiles
KT = D // 128        # 4 contraction chunks
NG = 8               # row-tile groups (8 r-tiles each) == column supertiles

F32 = mybir.dt.float32
BF16 = mybir.dt.bfloat16

_CACHE = {}


def _build_program():
    if "nc" in _CACHE:
        return _CACHE["nc"]
    nc = bacc.Bacc(
        "TRN2",
        target_bir_lowering=False,
        debug=False,
        num_devices=NCORES,
    )

    z = nc.dram_tensor("z", [N, D], BF16, kind="ExternalInput").ap()
    ident = nc.dram_tensor("ident", [128, 128], BF16, kind="ExternalInput").ap()
    negeye = nc.dram_tensor("negeye", [128, 128], F32, kind="ExternalInput").ap()

    mx_d = nc.dram_tensor("mx", [128, MT], F32, kind="ExternalOutput").ap()
    esum_d = nc.dram_tensor("esum", [128, MT, NG], F32, kind="ExternalOutput").ap()
    posd_d = nc.dram_tensor("posd", [128, MT], F32, kind="ExternalOutput").ap()
    ssq_d = nc.dram_tensor("ssq", [128, RT], F32, kind="ExternalOutput").ap()

    ALU = mybir.AluOpType
    AF = mybir.ActivationFunctionType
    AX = mybir.AxisListType

    with tile.TileContext(nc) as tc:
        with (
            tc.tile_pool(name="persist", bufs=1) as persist,
            tc.tile_pool(name="nrows", bufs=4) as nrows,
            tc.tile_pool(name="sqtr", bufs=2) as sqtrp,
            tc.tile_pool(name="etodd", bufs=6) as etoddp,
            tc.tile_pool(name="pstr", bufs=2, space="PSUM") as pstrp,
            tc.tile_pool(name="mm", bufs=3, space="PSUM") as mmp,
        ):
            zfull = persist.tile([128, RT, 512], BF16, tag="zfull")
            # rblk-major transposed reps:
            # repsT[p, rblk, sub, k, c] = feature k*128+p of local row
            #   (rblk*2+sub)*128 + c
            repsT = persist.tile([128, RT // 2, 2, KT, 128], BF16, tag="repsT")
            identS = persist.tile([128, 128], BF16, tag="identS")
            negeyeS = persist.tile([128, 128], F32, tag="negeyeS")
            ssqall = persist.tile([128, RT], F32, tag="ssqall")
            lnssq = persist.tile([128, RT], F32, tag="lnssq")
            invall = persist.tile([128, RT], F32, tag="invall")
            posdt = persist.tile([128, MT], F32, tag="posdt")
            mxf = persist.tile([128, MT], F32, tag="mxf")
            esm = persist.tile([128, MT, NG], F32, tag="esm")
            # G=0 exp tiles stay resident as the running max accumulator
            etev = persist.tile([128, MT, 1024], BF16, tag="etev")
            negfour = persist.tile([128, 1], F32, tag="negfour")

            nc.vector.memset(negfour, -4.0)
            nc.vector.memset(invall, 1.0 / float(np.sqrt(D)))
            warm = persist.tile([128, 1], F32, tag="warm")
            # load the exp activation table off the critical path
            nc.scalar.activation(warm, negfour, AF.Exp)

            def prep_dma(g, split=False):
                parts = ((0, 4), (4, 8)) if split else ((0, 8),)
                for lo, hi in parts:
                    nc.sync.dma_start(
                        out=zfull[:, g * 8 + lo : g * 8 + hi, :],
                        in_=z[
                            g * 1024 + lo * 128 : g * 1024 + hi * 128, :
                        ].rearrange("(j p) f -> p j f", p=128),
                    )

            def prep_head_span(g, nrow, lo, hi):
                """ssq + inv-norm + normalized rows for r-tiles [lo,hi) of g."""
                gs = slice(g * 8 + lo, g * 8 + hi)
                for r in range(g * 8 + lo, g * 8 + hi):
                    sq = sqtrp.tile([128, 512], BF16, tag="sqtr")
                    nc.vector.scalar_tensor_tensor(
                        out=sq,
                        in0=zfull[:, r, :],
                        scalar=1.0,
                        in1=zfull[:, r, :],
                        op0=ALU.mult,
                        op1=ALU.mult,
                        accum_out=ssqall[:, r : r + 1],
                    )
                # inv = rsqrt(ssq) via Newton on DVE (no ScalarE table
                # switches). Rows are randn[512]: ssq concentrates near 512,
                # so the constant seed 1/sqrt(512) converges quadratically;
                # 3 iters -> ~1e-9 rel err.
                w = hi - lo
                t1 = sqtrp.tile([128, 8], F32, tag="nwt")
                iv = invall[:, gs]
                for _ in range(3):
                    nc.vector.tensor_mul(t1[:, :w], iv, iv)
                    nc.vector.tensor_mul(t1[:, :w], t1[:, :w], ssqall[:, gs])
                    nc.vector.tensor_scalar(
                        out=t1[:, :w],
                        in0=t1[:, :w],
                        scalar1=-0.5,
                        scalar2=1.5,
                        op0=ALU.mult,
                        op1=ALU.add,
                    )
                    nc.vector.tensor_mul(iv, iv, t1[:, :w])
                for j in range(lo, hi):
                    r = g * 8 + j
                    nc.vector.tensor_scalar_mul(
                        nrow[:, j, :], zfull[:, r, :], invall[:, r : r + 1]
                    )

            def prep_pe(g, halves=False):
                """groups for the pipeline head: PE transpose + DVE copy,
                pipelined per half-group so matmuls can start early."""
                nrow = nrows.tile([128, 8, 512], BF16, tag="nrow")
                spans = ((0, 4), (4, 8)) if halves else ((0, 8),)
                for lo, hi in spans:
                    prep_head_span(g, nrow, lo, hi)
                    for j in range(lo, hi):
                        r = g * 8 + j
                        pstr = pstrp.tile([128, KT, 128], BF16, tag="pstr")
                        for k in range(KT):
                            nc.tensor.transpose(
                                pstr[:, k, :],
                                nrow[:, j, k * 128 : (k + 1) * 128],
                                identS,
                            )
                        nc.vector.tensor_copy(
                            out=repsT[:, r // 2, r % 2, :, :], in_=pstr
                        )

            def prep_xbar(g, lo=0, hi=8):
                """steady-state groups: one batched XBAR DMA-transpose
                per span (halves interleave with the G0 mask chain)."""
                nrow = nrows.tile([128, 8, 512], BF16, tag="nrow")
                prep_head_span(g, nrow, lo, hi)
                nc.sync.dma_start(
                    out=repsT[:, g * 4 + lo // 2 : g * 4 + hi // 2, :, :, :],
                    in_=nrow[:, lo:hi, :],
                    transpose=True,
                )

            def positives():
                for q in range(MT):
                    sq = sqtrp.tile([128, 512], BF16, tag="sqtr")
                    nc.vector.scalar_tensor_tensor(
                        out=sq,
                        in0=zfull[:, q, :],
                        scalar=1.0,
                        in1=zfull[:, 32 + q, :],
                        op0=ALU.mult,
                        op1=ALU.mult,
                        accum_out=posdt[:, q : q + 1],
                    )

            def main_m(G, m):
                ps = mmp.tile([128, 1024], F32, tag="ps")
                for h in (0, 1):
                    for k in range(KT):
                        nc.tensor.matmul(
                            ps[:, h * 512 : (h + 1) * 512],
                            lhsT=repsT[:, m // 2, m % 2, k, :],
                            rhs=repsT[:, 4 * G + 2 * h : 4 * G + 2 * h + 2, :, k, :],
                            start=(k == 0),
                            stop=(k == KT - 1),
                        )
                if G == 0:
                    # mask self-similarity: sim[p, m*128+p] -= 1e30
                    nc.vector.tensor_add(
                        ps[:, m * 128 : (m + 1) * 128],
                        ps[:, m * 128 : (m + 1) * 128],
                        negeyeS,
                    )
                if G == 0:
                    et = etev[:, m, :]
                else:
                    et = etoddp.tile([128, 1024], BF16, tag="etodd")
                nc.scalar.activation(
                    out=et,
                    in_=ps,
                    func=AF.Exp,
                    bias=negfour,
                    scale=4.0,
                    accum_out=esm[:, m, G : G + 1],
                )
                if G > 0:
                    # running elementwise max into the resident G=0 tile
                    nc.vector.tensor_max(etev[:, m, :], etev[:, m, :], et)
                if G == NG - 1:
                    nc.vector.reduce_max(mxf[:, m : m + 1], etev[:, m, :], axis=AX.X)

            # ---- schedule ----
            prep_dma(0, split=True)
            nc.sync.dma_start(out=identS, in_=ident)
            nc.sync.dma_start(out=negeyeS, in_=negeye)
            prep_dma(1)
            prep_pe(0, halves=True)
            for g in range(2, NG):
                prep_dma(g)
            for G in range(NG):
                for m in range(MT):
                    main_m(G, m)
                    if G == 0 and m == 2:
                        prep_xbar(1, 0, 4)
                    if G == 0 and m == 4:
                        prep_xbar(1, 4, 8)
                    if G == 0 and m == 6:
                        prep_xbar(2, 0, 4)
                    if G == 1 and m == 1:
                        prep_xbar(2, 4, 8)
                    if 1 <= G < 6 and m == 3:
                        prep_xbar(G + 2)
                    if G == 6 and m == 1:
                        positives()
                if G == 6:
                    nc.sync.dma_start(out=posd_d, in_=posdt)
                    nc.sync.dma_start(out=ssq_d, in_=ssqall)
                if G == 5:
                    nc.sync.dma_start(
                        out=esum_d[:, :, : NG - 2], in_=esm[:, :, : NG - 2]
                    )

            nc.sync.dma_start(out=mx_d, in_=mxf)
            nc.sync.dma_start(
                out=esum_d[:, :, NG - 2 :], in_=esm[:, :, NG - 2 :]
            )

    nc.compile()
    _CACHE["nc"] = nc
    return nc


def _host_inputs(z_i, z_j):
    reps = np.concatenate(
        [np.asarray(z_i, np.float32), np.asarray(z_j, np.float32)], axis=0
    )
    zb = reps.astype(ml_dtypes.bfloat16)
    ident = np.eye(128, dtype=np.float32).astype(ml_dtypes.bfloat16)
    negeye = (np.eye(128, dtype=np.float32) * -1.0e30).astype(np.float32)
    in_maps = []
    for c in range(NCORES):
        zc = np.ascontiguousarray(np.roll(zb, -c * NLOC, axis=0))
        in_maps.append({"z": zc, "ident": ident, "negeye": negeye})
    return in_maps


def _combine(results):
    pos = np.zeros(N, np.float64)
    hn = np.zeros(N, np.float64)
    S = 0.0
    for c, o in enumerate(results):
        mx = np.asarray(o["mx"], np.float64)       # [128, MT]
        esum = np.asarray(o["esum"], np.float64)   # [128, MT, NG]
        posd = np.asarray(o["posd"], np.float64)   # [128, MT]
        ssq = np.asarray(o["ssq"], np.float64)     # [128, RT]
        # mx holds max over exp(4*sim-4) (bf16 rounded); invert the exp.
        hn_loc = (np.log(mx.T.reshape(NLOC)) + 4.0) / 4.0
        S += esum.sum()                            # self terms exp'd to 0
        invrow = 1.0 / np.sqrt(ssq.T.reshape(N))   # rolled row index
        posl = posd.T.reshape(NLOC) * invrow[:NLOC] * invrow[B : B + NLOC]
        gl = (np.arange(NLOC) + c * NLOC) % N
        pos[gl] = posl
        hn[gl] = hn_loc
    ce = np.mean(np.logaddexp(0.0, 40.0 * hn - 20.0 * pos))
    npairs = N * (N - 1) // 2
    uniformity = np.log(S / 2.0 / npairs)
    return np.array(ce + 0.2 * uniformity, dtype=np.float32)


def run(z_i, z_j, **spmd_kwargs):
    nc = _build_program()
    in_maps = _host_inputs(z_i, z_j)
    res = run_bass_kernel_spmd(nc, in_maps, core_ids=list(range(NCORES)), **spmd_kwargs)
    return _combine(res.results), res


def kernel(z_i, z_j):
    loss, _ = run(z_i, z_j)
    return loss



# revision 2
# speedup vs baseline: 1.5519x; 1.5519x over previous
"""NT-Xent loss kernel for 8 Trainium2 NeuronCores (Bass/Tile).

Strategy (data-parallel rows, SPMD, fp8 DoubleRow matmul):
  - Host: L2-normalize rows of concat(z_i, z_j) in f64, scale by 16, cast
    to fp8 e4m3, and pack TRANSPOSED as zt[p, kp, ks, col] where feature
    k = kp*256 + ks*128 + p (DoubleRow contracts 2 k-planes of 128 per
    pass at 2 MACs/cell/cycle => ~1.8x bf16 matmul throughput; simulated
    end-to-end fp8 loss error ~6e-4, far under the 2e-2 gate).
    Core c gets np.roll(zt, -c*1024, axis=3): its 1024 rows sit at
    columns 0-1023, so one static program serves all cores.
  - Device per core: sim block [1024, 8192] via fp8 DoubleRow matmuls
    into [128, 2048] PSUM tiles (weights reloaded once per 2048-col
    sweep so LDWEIGHTS hides behind streaming). Self-column masked with
    -1e30 eye. exp(4*sim-4) on ScalarE with fused row-sum accumulation
    (ScalarE is the bottleneck: 1 elem/cycle @1.2GHz conversion+exp).
    Row-max via running elementwise tensor_max on DVE (bf16 2x mode) +
    per-m fold+reduce.
  - Host: positives exactly from f64 normalized reps; combine per-core
    max/esum stats into the scalar loss in f64.
"""

import numpy as np
import ml_dtypes

import concourse.bacc as bacc
import concourse.bass as bass
import concourse.tile as tile
import concourse.mybir as mybir
from concourse.bass_utils import run_bass_kernel_spmd

B = 4096
D = 512
N = 2 * B            # 8192 rows total
NCORES = 8
NLOC = N // NCORES   # 1024 rows per core
MT = NLOC // 128     # 8 local row tiles
QT = 4               # column quarters of 2048
SCALE = 16.0         # fp8 pre-quantization scale
ESC = 4.0 / (SCALE * SCALE)  # activation scale: 4*sim = ESC * psum

F32 = mybir.dt.float32
BF16 = mybir.dt.bfloat16
FP8 = mybir.dt.float8e4
DR = mybir.MatmulPerfMode.DoubleRow

_CACHE = {}


def _build_program():
    if "nc" in _CACHE:
        return _CACHE["nc"]
    nc = bacc.Bacc(
        "TRN2",
        target_bir_lowering=False,
        debug=False,
        num_devices=NCORES,
    )

    zt = nc.dram_tensor("zt", [128, 2, 2, N], FP8, kind="ExternalInput").ap()
    negeye = nc.dram_tensor("negeye", [128, 128], F32, kind="ExternalInput").ap()

    mx_d = nc.dram_tensor("mx", [128, MT], F32, kind="ExternalOutput").ap()
    esum_d = nc.dram_tensor("esum", [128, MT, QT], F32, kind="ExternalOutput").ap()

    ALU = mybir.AluOpType
    AF = mybir.ActivationFunctionType
    AX = mybir.AxisListType

    with tile.TileContext(nc) as tc:
        with (
            tc.tile_pool(name="persist", bufs=1) as persist,
            tc.tile_pool(name="escr", bufs=3) as escr,
            tc.tile_pool(name="fold", bufs=2) as foldp,
            tc.tile_pool(name="mm", bufs=2, space="PSUM") as mmp,
        ):
            ztS = persist.tile([128, 2, 2, N], FP8, tag="ztS")
            negeyeS = persist.tile([128, 128], F32, tag="negeyeS")
            # running elementwise max over column quarters, per m-tile
            etev = persist.tile([128, MT, 2048], BF16, tag="etev")
            esm = persist.tile([128, MT, QT], F32, tag="esm")
            mxf = persist.tile([128, MT], F32, tag="mxf")
            negfour = persist.tile([128, 1], F32, tag="negfour")

            nc.vector.memset(negfour, -4.0)
            warm = persist.tile([128, 1], F32, tag="warm")
            # load the exp activation table off the critical path
            nc.scalar.activation(warm, negfour, AF.Exp)
            nc.sync.dma_start(out=negeyeS, in_=negeye)

            def chunk(i):
                nc.sync.dma_start(
                    out=ztS[:, :, :, i * 1024 : (i + 1) * 1024],
                    in_=zt[:, :, :, i * 1024 : (i + 1) * 1024],
                )

            chunk(0)
            chunk(1)

            for q in range(QT):
                for m in range(MT):
                    ps = mmp.tile([128, 2048], F32, tag="ps")
                    for kp in range(2):
                        for h in range(4):
                            c0 = q * 2048 + h * 512
                            nc.tensor.matmul(
                                ps[:, h * 512 : (h + 1) * 512],
                                lhsT=ztS[:, kp, :, m * 128 : (m + 1) * 128],
                                rhs=ztS[:, kp, :, c0 : c0 + 512],
                                start=(kp == 0),
                                stop=(kp == 1),
                                perf_mode=DR,
                            )
                    if q == 0:
                        # mask self-similarity: sim[p, m*128+p] -= 1e30
                        nc.vector.tensor_add(
                            ps[:, m * 128 : (m + 1) * 128],
                            ps[:, m * 128 : (m + 1) * 128],
                            negeyeS,
                        )
                        nc.scalar.activation(
                            out=etev[:, m, :],
                            in_=ps,
                            func=AF.Exp,
                            bias=negfour,
                            scale=ESC,
                            accum_out=esm[:, m, 0:1],
                        )
                        # stage remaining input chunks during q0 compute
                        if 2 <= m:
                            chunk(m)
                    else:
                        et = escr.tile([128, 2048], BF16, tag="et")
                        nc.scalar.activation(
                            out=et,
                            in_=ps,
                            func=AF.Exp,
                            bias=negfour,
                            scale=ESC,
                            accum_out=esm[:, m, q : q + 1],
                        )
                        nc.vector.tensor_max(etev[:, m, :], etev[:, m, :], et)
                    if q == QT - 1:
                        mhalf = foldp.tile([128, 1024], BF16, tag="mhalf")
                        nc.vector.tensor_max(
                            mhalf, etev[:, m, 0:1024], etev[:, m, 1024:2048]
                        )
                        nc.vector.reduce_max(mxf[:, m : m + 1], mhalf, axis=AX.X)

            nc.sync.dma_start(out=mx_d, in_=mxf)
            nc.sync.dma_start(out=esum_d, in_=esm)

    nc.compile()
    _CACHE["nc"] = nc
    return nc


def _host_inputs(z_i, z_j):
    reps = np.concatenate(
        [np.asarray(z_i, np.float64), np.asarray(z_j, np.float64)], axis=0
    )
    nrm = np.maximum(np.sqrt(np.sum(reps * reps, axis=1, keepdims=True)), 1e-12)
    reps_n = reps / nrm
    # positives (exact, f64): row i of z_i with row i of z_j
    pos_half = np.sum(reps_n[:B] * reps_n[B:], axis=1)
    pos = np.concatenate([pos_half, pos_half])

    scaled = (reps_n * SCALE).astype(np.float32).astype(ml_dtypes.float8_e4m3)
    # zt[p, kp, ks, col] = scaled[col, kp*256 + ks*128 + p]
    zt0 = np.ascontiguousarray(
        scaled.T.reshape(2, 2, 128, N).transpose(2, 0, 1, 3)
    )
    negeye = (np.eye(128, dtype=np.float32) * -1.0e30).astype(np.float32)
    in_maps = []
    for c in range(NCORES):
        ztc = np.ascontiguousarray(np.roll(zt0, -c * NLOC, axis=3))
        in_maps.append({"zt": ztc, "negeye": negeye})
    return in_maps, pos


def _combine(results, pos):
    hn = np.zeros(N, np.float64)
    S = 0.0
    for c, o in enumerate(results):
        mx = np.asarray(o["mx"], np.float64)       # [128, MT]
        esum = np.asarray(o["esum"], np.float64)   # [128, MT, QT]
        # mx holds max over exp(4*sim-4) (bf16 rounded); invert the exp.
        hn_loc = (np.log(mx.T.reshape(NLOC)) + 4.0) / 4.0
        S += esum.sum()                            # self terms exp'd to 0
        gl = (np.arange(NLOC) + c * NLOC) % N
        hn[gl] = hn_loc
    ce = np.mean(np.logaddexp(0.0, 40.0 * hn - 20.0 * pos))
    npairs = N * (N - 1) // 2
    uniformity = np.log(S / 2.0 / npairs)
    return np.array(ce + 0.2 * uniformity, dtype=np.float32)


def run(z_i, z_j, **spmd_kwargs):
    nc = _build_program()
    in_maps, pos = _host_inputs(z_i, z_j)
    res = run_bass_kernel_spmd(nc, in_maps, core_ids=list(range(NCORES)), **spmd_kwargs)
    return _combine(res.results, pos), res


def kernel(z_i, z_j):
    loss, _ = run(z_i, z_j)
    return loss


# revision 3
# speedup vs baseline: 1.8571x; 1.1967x over previous
"""NT-Xent loss kernel for 8 Trainium2 NeuronCores (Bass/Tile).

Strategy (symmetric data-parallel, SPMD, fp8 DoubleRow matmul):
  - Host: L2-normalize rows of concat(z_i, z_j) in f64, scale by 16, cast
    to fp8 e4m3, pack TRANSPOSED as zt[p, kp, ks, col] (feature
    k = kp*256 + ks*128 + p; DoubleRow contracts 2 k-planes per pass).
    Core c gets the rolled column window [c*1024, c*1024 + 5*1024) so its
    1024 rows sit at local columns 0-1023.
  - Symmetry: core c computes only column groups G0..G4 (5/8 of the sim
    matrix). Ordered-pair bookkeeping on host: G0 entries weight 1 (both
    orders inside the block), G1-3 weight 2 (reverse order never
    computed), G4 weight 1 (partner core computes the transposed block).
    Hard negatives for skipped blocks come from COLUMN maxes of G1-3,
    accumulated on-device as elementwise-max tiles and partition-reduced
    on the host.
  - Device per core: fp8 DoubleRow matmuls into [128, 2048] PSUM tiles;
    self-diag masked with -1e30 eye; exp(4*sim-4) on ScalarE with fused
    row-sum accumulation (ScalarE is the pacer at ~47us); DVE keeps a
    1024-wide running row-max plus G1/G23 column-max accumulators.
  - Host: positives exactly from f64 normalized reps; row-max reduce of
    the [128, 8, 1024] max tiles; column-max partition reduce; weighted
    esum -> uniformity; combine into the scalar loss in f64.
"""

import numpy as np
import ml_dtypes

import concourse.bacc as bacc
import concourse.bass as bass
import concourse.tile as tile
import concourse.mybir as mybir
from concourse.bass_utils import run_bass_kernel_spmd

B = 4096
D = 512
N = 2 * B            # 8192 rows total
NCORES = 8
NLOC = N // NCORES   # 1024 rows per core
MT = NLOC // 128     # 8 local row tiles
NG = 5               # column groups computed per core (G0..G4)
NCOL = NG * 1024     # 5120 columns per core
SCALE = 16.0         # fp8 pre-quantization scale
ESC = 4.0 / (SCALE * SCALE)  # activation scale: 4*sim = ESC * psum

F32 = mybir.dt.float32
BF16 = mybir.dt.bfloat16
FP8 = mybir.dt.float8e4
DR = mybir.MatmulPerfMode.DoubleRow

_CACHE = {}


def _build_program():
    if "nc" in _CACHE:
        return _CACHE["nc"]
    nc = bacc.Bacc(
        "TRN2",
        target_bir_lowering=False,
        debug=False,
        num_devices=NCORES,
    )

    zt = nc.dram_tensor("zt", [128, 2, 2, NCOL], FP8, kind="ExternalInput").ap()
    negeye = nc.dram_tensor("negeye", [128, 128], F32, kind="ExternalInput").ap()

    # row-side max of exp(4 sim - 4) over each m's 1024-wide running tile
    mx_d = nc.dram_tensor("mx", [128, MT, 1024], BF16, kind="ExternalOutput").ap()
    # esum slots: 0=G0, 1=G1, 2=G2+G3, 3=G4
    esum_d = nc.dram_tensor("esum", [128, MT, 4], F32, kind="ExternalOutput").ap()
    # column-max accumulators: [c1e | c1o | c23e | c23o]
    cacc_d = nc.dram_tensor("cacc", [128, 6144], BF16, kind="ExternalOutput").ap()

    ALU = mybir.AluOpType
    AF = mybir.ActivationFunctionType
    AX = mybir.AxisListType

    with tile.TileContext(nc) as tc:
        with (
            tc.tile_pool(name="persist", bufs=1) as persist,
            tc.tile_pool(name="escr", bufs=3) as escr,
            tc.tile_pool(name="mm", bufs=2, space="PSUM") as mmp,
        ):
            ztS = persist.tile([128, 2, 2, NCOL], FP8, tag="ztS")
            negeyeS = persist.tile([128, 128], F32, tag="negeyeS")
            etev = persist.tile([128, MT, 1024], BF16, tag="etev")
            esm = persist.tile([128, MT, 4], F32, tag="esm")
            # column-max accumulators (even/odd m for tail overlap)
            c1e = persist.tile([128, 1024], BF16, tag="c1e")
            c1o = persist.tile([128, 1024], BF16, tag="c1o")
            c23e = persist.tile([128, 2048], BF16, tag="c23e")
            c23o = persist.tile([128, 2048], BF16, tag="c23o")
            negfour = persist.tile([128, 1], F32, tag="negfour")

            nc.vector.memset(negfour, -4.0)
            warm = persist.tile([128, 1], F32, tag="warm")
            nc.scalar.activation(warm, negfour, AF.Exp)
            nc.sync.dma_start(out=negeyeS, in_=negeye)

            def chunk(lo, hi):
                nc.sync.dma_start(
                    out=ztS[:, :, :, lo:hi], in_=zt[:, :, :, lo:hi]
                )

            chunk(0, 512)
            chunk(512, 1024)
            chunk(1024, 2048)

            def mms(ps, m, c0, nh):
                """DoubleRow matmuls: psum[:, :nh*512] = sim block
                [m-tile rows x cols c0:c0+nh*512] (scaled)."""
                for kp in range(2):
                    for h in range(nh):
                        nc.tensor.matmul(
                            ps[:, h * 512 : (h + 1) * 512],
                            lhsT=ztS[:, kp, :, m * 128 : (m + 1) * 128],
                            rhs=ztS[:, kp, :, c0 + h * 512 : c0 + (h + 1) * 512],
                            start=(kp == 0),
                            stop=(kp == 1),
                            perf_mode=DR,
                        )

            for m in range(MT):
                cacc1 = c1e if m % 2 == 0 else c1o
                cacc23 = c23e if m % 2 == 0 else c23o
                # --- t0: G0 + G1 (cols 0..2047)
                ps0 = mmp.tile([128, 2048], F32, tag="ps")
                mms(ps0, m, 0, 4)
                nc.vector.tensor_add(
                    ps0[:, m * 128 : (m + 1) * 128],
                    ps0[:, m * 128 : (m + 1) * 128],
                    negeyeS,
                )
                nc.scalar.activation(
                    out=etev[:, m, :], in_=ps0[:, 0:1024], func=AF.Exp,
                    bias=negfour, scale=ESC, accum_out=esm[:, m, 0:1],
                )
                et1 = escr.tile([128, 1024], BF16, tag="et1")
                nc.scalar.activation(
                    out=et1, in_=ps0[:, 1024:2048], func=AF.Exp,
                    bias=negfour, scale=ESC, accum_out=esm[:, m, 1:2],
                )
                if m < 2:
                    nc.vector.tensor_copy(out=cacc1, in_=et1)
                else:
                    nc.vector.tensor_max(cacc1, cacc1, et1)
                nc.vector.tensor_max(etev[:, m, :], etev[:, m, :], et1)
                # stage later input chunks behind the first compute
                if m == 0:
                    chunk(2048, 3072)
                    chunk(3072, 4096)
                    chunk(4096, 5120)
                # --- t1: G2 + G3 (cols 2048..4095)
                ps1 = mmp.tile([128, 2048], F32, tag="ps")
                mms(ps1, m, 2048, 4)
                et23 = escr.tile([128, 2048], BF16, tag="et23")
                nc.scalar.activation(
                    out=et23, in_=ps1, func=AF.Exp,
                    bias=negfour, scale=ESC, accum_out=esm[:, m, 2:3],
                )
                nc.vector.tensor_max(etev[:, m, :], etev[:, m, :], et23[:, 0:1024])
                nc.vector.tensor_max(etev[:, m, :], etev[:, m, :], et23[:, 1024:2048])
                if m < 2:
                    nc.vector.tensor_copy(out=cacc23, in_=et23)
                else:
                    nc.vector.tensor_max(cacc23, cacc23, et23)
                # --- t2: G4 (cols 4096..5119)
                ps2 = mmp.tile([128, 2048], F32, tag="ps")
                mms(ps2, m, 4096, 2)
                et4 = escr.tile([128, 1024], BF16, tag="et4")
                nc.scalar.activation(
                    out=et4, in_=ps2[:, 0:1024], func=AF.Exp,
                    bias=negfour, scale=ESC, accum_out=esm[:, m, 3:4],
                )
                nc.vector.tensor_max(etev[:, m, :], etev[:, m, :], et4)
                nc.sync.dma_start(out=mx_d[:, m, :], in_=etev[:, m, :])
                if m == MT - 2:
                    # even-m accumulators are final; drain during m=7
                    nc.sync.dma_start(out=cacc_d[:, 0:1024], in_=c1e)
                    nc.sync.dma_start(out=cacc_d[:, 2048:4096], in_=c23e)

            nc.sync.dma_start(out=cacc_d[:, 1024:2048], in_=c1o)
            nc.sync.dma_start(out=cacc_d[:, 4096:6144], in_=c23o)
            nc.sync.dma_start(out=esum_d, in_=esm)

    nc.compile()
    _CACHE["nc"] = nc
    return nc


def _host_inputs(z_i, z_j):
    reps = np.concatenate(
        [np.asarray(z_i, np.float64), np.asarray(z_j, np.float64)], axis=0
    )
    nrm = np.maximum(np.sqrt(np.sum(reps * reps, axis=1, keepdims=True)), 1e-12)
    reps_n = reps / nrm
    pos_half = np.sum(reps_n[:B] * reps_n[B:], axis=1)
    pos = np.concatenate([pos_half, pos_half])

    scaled = (reps_n * SCALE).astype(np.float32).astype(ml_dtypes.float8_e4m3)
    # zt0[p, kp, ks, col] = scaled[col, kp*256 + ks*128 + p]
    zt0 = np.ascontiguousarray(
        scaled.T.reshape(2, 2, 128, N).transpose(2, 0, 1, 3)
    )
    ztw = np.concatenate([zt0, zt0[:, :, :, : NCOL - 1024]], axis=3)
    negeye = (np.eye(128, dtype=np.float32) * -1.0e30).astype(np.float32)
    in_maps = []
    for c in range(NCORES):
        ztc = np.ascontiguousarray(ztw[:, :, :, c * NLOC : c * NLOC + NCOL])
        in_maps.append({"zt": ztc, "negeye": negeye})
    return in_maps, pos


def _combine(results, pos):
    hn = np.full(N, -np.inf)
    S = 0.0
    for c, o in enumerate(results):
        mx = np.asarray(o["mx"], np.float32)       # [128, MT, 1024] bf16->f32
        esum = np.asarray(o["esum"], np.float64)   # [128, MT, 4]
        cacc = np.asarray(o["cacc"], np.float32)   # [128, 6144]
        # row-side: max over the 1024-wide running tile
        hn_loc = mx.max(axis=2).T.reshape(NLOC)    # local rows m*128+p
        gl = (np.arange(NLOC) + c * NLOC) % N
        np.maximum.at(hn, gl, hn_loc)
        # esum weights: G0=1, G1=2, G2+G3=2, G4=1
        es = esum.sum(axis=(0, 1))
        S += es[0] + 2.0 * es[1] + 2.0 * es[2] + es[3]
        # column-side maxes for G1..G3 (partition reduce on host)
        cm1 = np.maximum(cacc[:, 0:1024], cacc[:, 1024:2048]).max(axis=0)
        cm23 = np.maximum(cacc[:, 2048:4096], cacc[:, 4096:6144]).max(axis=0)
        g1 = (np.arange(1024) + c * NLOC + 1024) % N
        g2 = (np.arange(1024) + c * NLOC + 2048) % N
        g3 = (np.arange(1024) + c * NLOC + 3072) % N
        np.maximum.at(hn, g1, cm1)
        np.maximum.at(hn, g2, cm23[0:1024])
        np.maximum.at(hn, g3, cm23[1024:2048])
    # hn holds max of exp(4*sim-4) (bf16 rounded); invert the exp.
    hn = (np.log(hn.astype(np.float64)) + 4.0) / 4.0
    ce = np.mean(np.logaddexp(0.0, 40.0 * hn - 20.0 * pos))
    npairs = N * (N - 1) // 2
    uniformity = np.log(S / 2.0 / npairs)
    return np.array(ce + 0.2 * uniformity, dtype=np.float32)


def run(z_i, z_j, **spmd_kwargs):
    nc = _build_program()
    in_maps, pos = _host_inputs(z_i, z_j)
    res = run_bass_kernel_spmd(nc, in_maps, core_ids=list(range(NCORES)), **spmd_kwargs)
    return _combine(res.results, pos), res


def kernel(z_i, z_j):
    loss, _ = run(z_i, z_j)
    return loss


# revision 6
# speedup vs baseline: 2.1118x; 1.1372x over previous
"""NT-Xent loss kernel for 8 Trainium2 NeuronCores (Bass/Tile).

Strategy (symmetric data-parallel, SPMD, fp8 DoubleRow matmul):
  - Host: L2-normalize rows of concat(z_i, z_j) in f64, scale by 16, cast
    to fp8 e4m3, pack TRANSPOSED as zt[p, kp, ks, col] (feature
    k = kp*256 + ks*128 + p; DoubleRow contracts 2 k-planes per pass at
    ~1.8x bf16 matmul throughput; end-to-end fp8 loss error ~8e-4 vs the
    2e-2 gate). Core c gets the rolled column window
    [c*1024, c*1024 + 5*1024) so its 1024 rows sit at local cols 0-1023.
  - Symmetry: core c computes only column groups G0..G4 (5/8 of the sim
    matrix). Ordered-pair bookkeeping on host: G0 entries weight 1, G1-3
    weight 2 (reverse order never computed), G4 weight 1 (partner core
    computes the transposed block). Hard negatives for skipped blocks
    come from COLUMN maxes of G1-3, accumulated on-device as
    elementwise-max tiles and partition-reduced on the host.
  - Device: ScalarE exp(4*sim-4) with fused row-sum accum is the pacer
    (~46us). Phases: A = G0+G1 for all m, then B+C interleaved =
    G2G3 + G4 per m, so Scalar stays saturated. PSUM is a manually
    rotated 8x[128,1024] ring giving the PE multiple m of lookahead.
    DVE keeps a 1024-wide running row-max (DMA'd out per m, reduced on
    host) plus G1/G23 column-max accumulators (even/odd m split so the
    even half drains early).
  - Host: positives exactly from f64 normalized reps; row-max reduce;
    column partition-max; weighted esum -> uniformity; f64 combine.
"""

import numpy as np
import ml_dtypes

import concourse.bacc as bacc
import concourse.bass as bass
import concourse.tile as tile
import concourse.mybir as mybir
from concourse.bass_utils import run_bass_kernel_spmd

B = 4096
D = 512
N = 2 * B            # 8192 rows total
NCORES = 8
NLOC = N // NCORES   # 1024 rows per core
MT = NLOC // 128     # 8 local row tiles
NG = 5               # column groups computed per core (G0..G4)
NCOL = NG * 1024     # 5120 columns per core
SCALE = 16.0         # fp8 pre-quantization scale
ESC = 4.0 / (SCALE * SCALE)  # activation scale: 4*sim = ESC * psum

F32 = mybir.dt.float32
BF16 = mybir.dt.bfloat16
FP8 = mybir.dt.float8e4
DR = mybir.MatmulPerfMode.DoubleRow

_CACHE = {}


def _build_program():
    if "nc" in _CACHE:
        return _CACHE["nc"]
    nc = bacc.Bacc(
        "TRN2",
        target_bir_lowering=False,
        debug=False,
        num_devices=NCORES,
    )

    zt = nc.dram_tensor("zt", [128, 2, 2, NCOL], FP8, kind="ExternalInput").ap()
    negeye = nc.dram_tensor("negeye", [128, 128], F32, kind="ExternalInput").ap()

    # row-side running max of exp(4 sim - 4), 1024-wide per m
    mx_d = nc.dram_tensor("mx", [128, MT, 1024], BF16, kind="ExternalOutput").ap()
    # esum slots: 0=G0, 1=G1, 2=G2+G3, 3=G4
    esum_d = nc.dram_tensor("esum", [128, MT, 4], F32, kind="ExternalOutput").ap()
    # column-max accumulators: [c1e | c1o | c23e | c23o]
    cacc_d = nc.dram_tensor("cacc", [128, 6144], BF16, kind="ExternalOutput").ap()

    ALU = mybir.AluOpType
    AF = mybir.ActivationFunctionType

    with tile.TileContext(nc) as tc:
        with (
            tc.tile_pool(name="persist", bufs=1) as persist,
            tc.tile_pool(name="escr", bufs=3) as escr,
            tc.tile_pool(name="ring", bufs=1, space="PSUM") as ringp,
        ):
            ztS = persist.tile([128, 2, 2, NCOL], FP8, tag="ztS")
            negeyeS = persist.tile([128, 128], F32, tag="negeyeS")
            etev = persist.tile([128, MT, 1024], BF16, tag="etev")
            esm = persist.tile([128, MT, 4], F32, tag="esm")
            c1e = persist.tile([128, 1024], BF16, tag="c1e")
            c1o = persist.tile([128, 1024], BF16, tag="c1o")
            c23e = persist.tile([128, 2048], BF16, tag="c23e")
            c23o = persist.tile([128, 2048], BF16, tag="c23o")
            negfour = persist.tile([128, 1], F32, tag="negfour")
            ring = ringp.tile([128, 4, 1024], F32, tag="ring")

            nc.vector.memset(negfour, -4.0)
            warm = persist.tile([128, 1], F32, tag="warm")
            nc.scalar.activation(warm, negfour, AF.Exp)

            def chunk(lo, hi):
                nc.sync.dma_start(out=ztS[:, :, :, lo:hi], in_=zt[:, :, :, lo:hi])

            chunk(0, 512)
            nc.sync.dma_start(out=negeyeS, in_=negeye)
            chunk(512, 1024)
            chunk(1024, 2048)
            chunk(2048, 3072)
            chunk(3072, 4096)
            chunk(4096, 5120)

            def mms(qbase, m, c0, nh):
                """DoubleRow matmuls: ring regions qbase.. = sim block
                [m-tile rows x cols c0:c0+nh*512] (scaled by SCALE^2)."""
                for kp in range(2):
                    for h in range(nh):
                        nc.tensor.matmul(
                            ring[:, qbase + h // 2, (h % 2) * 512 : (h % 2) * 512 + 512],
                            lhsT=ztS[:, kp, :, m * 128 : (m + 1) * 128],
                            rhs=ztS[:, kp, :, c0 + h * 512 : c0 + (h + 1) * 512],
                            start=(kp == 0),
                            stop=(kp == 1),
                            perf_mode=DR,
                        )

            # --- Phase A: G0 + G1 for every m ---
            for m in range(MT):
                qb = (2 * m) % 4
                cacc1 = c1e if m % 2 == 0 else c1o
                mms(qb, m, 0, 4)
                nc.vector.tensor_add(
                    ring[:, qb, m * 128 : m * 128 + 128],
                    ring[:, qb, m * 128 : m * 128 + 128],
                    negeyeS,
                )
                nc.scalar.activation(
                    out=etev[:, m, :], in_=ring[:, qb, :], func=AF.Exp,
                    bias=negfour, scale=ESC, accum_out=esm[:, m, 0:1],
                )
                et1 = escr.tile([128, 1024], BF16, tag="et1")
                nc.scalar.activation(
                    out=et1, in_=ring[:, qb + 1, :], func=AF.Exp,
                    bias=negfour, scale=ESC, accum_out=esm[:, m, 1:2],
                )
                if m < 2:
                    nc.vector.tensor_copy(out=cacc1, in_=et1)
                else:
                    nc.vector.tensor_max(cacc1, cacc1, et1)
                nc.vector.tensor_max(etev[:, m, :], etev[:, m, :], et1)
                if m == MT - 2:
                    nc.sync.dma_start(out=cacc_d[:, 0:1024], in_=c1e)
            nc.sync.dma_start(out=cacc_d[:, 1024:2048], in_=c1o)

            # --- Phase B+C interleaved: G2G3 then G4 per m ---
            for m in range(MT):
                qb = 0
                q4 = 2 + m % 2
                cacc23 = c23e if m % 2 == 0 else c23o
                mms(qb, m, 2048, 4)       # G2, G3 -> regions 0, 1
                mms(q4, m, 4096, 2)       # G4 -> region 2 or 3
                et23 = escr.tile([128, 2048], BF16, tag="et23")
                nc.scalar.activation(
                    out=et23, in_=ring[:, qb : qb + 2, :], func=AF.Exp,
                    bias=negfour, scale=ESC, accum_out=esm[:, m, 2:3],
                )
                nc.vector.tensor_max(etev[:, m, :], etev[:, m, :], et23[:, 0:1024])
                nc.vector.tensor_max(etev[:, m, :], etev[:, m, :], et23[:, 1024:2048])
                if m < 2:
                    nc.vector.tensor_copy(out=cacc23, in_=et23)
                else:
                    nc.vector.tensor_max(cacc23, cacc23, et23)
                et4 = escr.tile([128, 1024], BF16, tag="et4")
                nc.scalar.activation(
                    out=et4, in_=ring[:, q4, :], func=AF.Exp,
                    bias=negfour, scale=ESC, accum_out=esm[:, m, 3:4],
                )
                nc.vector.tensor_max(etev[:, m, :], etev[:, m, :], et4)
                nc.sync.dma_start(out=mx_d[:, m, :], in_=etev[:, m, :])
                if m == MT - 2:
                    nc.sync.dma_start(out=cacc_d[:, 2048:4096], in_=c23e)

            nc.sync.dma_start(out=cacc_d[:, 4096:6144], in_=c23o)
            nc.sync.dma_start(out=esum_d, in_=esm)

    nc.compile()
    _CACHE["nc"] = nc
    return nc


def _host_inputs(z_i, z_j):
    reps = np.concatenate(
        [np.asarray(z_i, np.float64), np.asarray(z_j, np.float64)], axis=0
    )
    nrm = np.maximum(np.sqrt(np.sum(reps * reps, axis=1, keepdims=True)), 1e-12)
    reps_n = reps / nrm
    pos_half = np.sum(reps_n[:B] * reps_n[B:], axis=1)
    pos = np.concatenate([pos_half, pos_half])

    scaled = (reps_n * SCALE).astype(np.float32).astype(ml_dtypes.float8_e4m3)
    # zt0[p, kp, ks, col] = scaled[col, kp*256 + ks*128 + p]
    zt0 = np.ascontiguousarray(
        scaled.T.reshape(2, 2, 128, N).transpose(2, 0, 1, 3)
    )
    ztw = np.concatenate([zt0, zt0[:, :, :, : NCOL - 1024]], axis=3)
    negeye = (np.eye(128, dtype=np.float32) * -1.0e30).astype(np.float32)
    in_maps = []
    for c in range(NCORES):
        ztc = np.ascontiguousarray(ztw[:, :, :, c * NLOC : c * NLOC + NCOL])
        in_maps.append({"zt": ztc, "negeye": negeye})
    return in_maps, pos


def _combine(results, pos):
    hn = np.full(N, -np.inf)
    S = 0.0
    for c, o in enumerate(results):
        mx = np.asarray(o["mx"], np.float32)       # [128, MT, 1024]
        esum = np.asarray(o["esum"], np.float64)   # [128, MT, 4]
        cacc = np.asarray(o["cacc"], np.float32)   # [128, 6144]
        hn_loc = mx.max(axis=2).T.reshape(NLOC)    # local rows m*128+p
        gl = (np.arange(NLOC) + c * NLOC) % N
        np.maximum.at(hn, gl, hn_loc)
        es = esum.sum(axis=(0, 1))
        S += es[0] + 2.0 * es[1] + 2.0 * es[2] + es[3]
        cm1 = np.maximum(cacc[:, 0:1024], cacc[:, 1024:2048]).max(axis=0)
        cm23 = np.maximum(cacc[:, 2048:4096], cacc[:, 4096:6144]).max(axis=0)
        g1 = (np.arange(1024) + c * NLOC + 1024) % N
        g2 = (np.arange(1024) + c * NLOC + 2048) % N
        g3 = (np.arange(1024) + c * NLOC + 3072) % N
        np.maximum.at(hn, g1, cm1)
        np.maximum.at(hn, g2, cm23[0:1024])
        np.maximum.at(hn, g3, cm23[1024:2048])
    # hn holds max of exp(4*sim-4) (bf16 rounded); invert the exp.
    hn = (np.log(hn.astype(np.float64)) + 4.0) / 4.0
    ce = np.mean(np.logaddexp(0.0, 40.0 * hn - 20.0 * pos))
    npairs = N * (N - 1) // 2
    uniformity = np.log(S / 2.0 / npairs)
    return np.array(ce + 0.2 * uniformity, dtype=np.float32)


def run(z_i, z_j, **spmd_kwargs):
    nc = _build_program()
    in_maps, pos = _host_inputs(z_i, z_j)
    res = run_bass_kernel_spmd(nc, in_maps, core_ids=list(range(NCORES)), **spmd_kwargs)
    return _combine(res.results, pos), res


def kernel(z_i, z_j):
    loss, _ = run(z_i, z_j)
    return loss
